# revision 7
# baseline (speedup 1.0000x reference)
"""Trainium2 Bass kernel: per-point 3x3 Gaussian covariance from quaternion + log_scale.

cov = R diag(exp(log_scale)) R^T with R from the normalized quaternion.

Identity used (avoids normalizing q and the third rotation column):
  nu  = |q|^2 / 2
  C   = nu * R   (entries are plain quadratics of raw q: C00 = ha+hb-hc-hd,
                  C10 = bc+ad, C20 = bd-ac, C01 = bc-ad, C11 = ha-hb+hc-hd,
                  C21 = cd+ab, with hx = x^2/2)
  cov = s2*I + t0'*C0 C0^T + t1'*C1 C1^T,  tj' = (sj - s2)/nu^2
(uses R R^T = I to eliminate column 2.)

Layout: host uploads planar fp16 q[4, NPC], ls[3, NPC]; output is the 6
unique covariance entries planar fp16 cov6[6, NPC]; host symmetrizes and
upcasts.  On-chip everything is fp16 contiguous/block APs so every DVE
tensor_tensor hits the 2x_1p mode; squares/exp/copy run on ScalarE (all in
the one `exp_and_others` table set -> single ACT_TABLE_LOAD).  1/nu^2 via
the custom DVE reciprocal_approx_fast (no Ln, no table switch).
"""

import os
import numpy as np

import concourse.bass as bass
import concourse.bacc as bacc
import concourse.mybir as mybir
from concourse.tile import TileContext
from concourse.bass_utils import run_bass_kernel_spmd

AF = mybir.ActivationFunctionType
FP32 = mybir.dt.float32
FP16 = mybir.dt.float16

N_CORES = 8
N_FULL = 4_000_000
P = 128

F = int(os.environ.get("KERNEL_F", "978"))      # points per partition per tile
GPS = int(os.environ.get("KERNEL_GPSIMD", "1"))  # offload small ops to GPSIMD
NT = -(-3907 // F)                               # tiles so that P*R*8 >= N
R = F * NT                                       # rows per partition per core
NPC = P * R                                      # points per core (padded)

SQRT_HALF = 0.7071067811865476

_built = {}


def _apv(t, off, pairs):
    """Raw AP view of tile t: keep its partition dim, replace free dims.

    pairs = [[stride, count], ...] in elements, offset in elements from the
    tile's base.
    """
    ap = [list(p) for p in t.ap]
    return bass.AP(tensor=t.tensor, offset=t.offset + off, ap=[ap[0]] + pairs)


def _bc(ap2d, n):
    """[P, f] -> [P, n, f] broadcast (stride-0 middle dim)."""
    p, f = ap2d.shape
    return ap2d.unsqueeze(1).broadcast_to((p, n, f))


def _build():
    key = F
    if key in _built:
        return _built[key]

    nc = bacc.Bacc("TRN2", target_bir_lowering=False, debug=False, num_devices=N_CORES)
    q = nc.dram_tensor("q", [4, NPC], FP16, kind="ExternalInput")
    ls = nc.dram_tensor("ls", [3, NPC], FP16, kind="ExternalInput")
    cov = nc.dram_tensor("cov6", [6, NPC], FP16, kind="ExternalOutput")

    qv = q.ap().rearrange("c (p r) -> p c r", p=P)      # [P, 4, R]
    lsv = ls.ap().rearrange("c (p r) -> p c r", p=P)    # [P, 3, R]
    ov = cov.ap().rearrange("e (p r) -> p e r", p=P)    # [P, 6, R]

    with TileContext(nc) as tc:
        with (
            tc.tile_pool(name="io", bufs=2) as io,
            tc.tile_pool(name="wk", bufs=2) as wk,
        ):
            for it in range(NT):
                _tile_body(nc, io, wk, qv, lsv, ov, it * F, F)

    nc.compile()
    _built[key] = nc
    return nc


def _tile_body(nc, io, wk, qv, lsv, ov, t0, f):
    v = nc.vector
    s = nc.scalar

    def W(shape_f, dt=FP16, tag=None):
        return wk.tile([P, shape_f], dt, tag=tag, name=f"{tag}_{t0}")

    # ---- DMA in ----------------------------------------------------------
    q4 = io.tile([P, 4 * f], FP16, tag="q4", name=f"q4_{t0}")
    ls3 = io.tile([P, 3 * f], FP16, tag="ls3", name=f"ls3_{t0}")
    nc.sync.dma_start(out=q4.rearrange("p (c x) -> p c x", c=4),
                      in_=qv[:, :, t0:t0 + f])
    nc.sync.dma_start(out=ls3.rearrange("p (c x) -> p c x", c=3),
                      in_=lsv[:, :, t0:t0 + f])

    # ---- half-squares + nu + diagonal R entries --------------------------
    sq4 = W(4 * f, tag="sq4_pa")            # [ha|hb|hc|hd]
    s.activation(sq4, q4, AF.Square, scale=SQRT_HALF)

    pq = W(2 * f, tag="pq_pb")              # [ha-hc | hb-hd]
    st = W(2 * f, tag="st_tt")              # [ha+hc | hb+hd]
    v.tensor_sub(pq, sq4[:, 0:2 * f], sq4[:, 2 * f:4 * f])
    v.tensor_add(st, sq4[:, 0:2 * f], sq4[:, 2 * f:4 * f])

    r6 = W(6 * f, tag="r6")                 # [C00|C10|C20|C01|C11|C21]
    v.tensor_add(r6[:, 0:f], pq[:, 0:f], pq[:, f:2 * f])          # C00
    v.tensor_sub(r6[:, 4 * f:5 * f], pq[:, 0:f], pq[:, f:2 * f])  # C11
    nu = W(f, FP32, tag="nu")
    v.tensor_add(nu, st[:, 0:f], st[:, f:2 * f])

    # ---- raw quaternion products -----------------------------------------
    pp = W(3 * f, tag="pp")                 # [ab|ac|ad]
    qcd = W(3 * f, tag="qcd")               # [bc|bd|cd]
    q4c = q4.rearrange("p (c x) -> p c x", c=4)
    v.tensor_mul(pp.rearrange("p (c x) -> p c x", c=3),
                 _bc(q4[:, 0:f], 3), q4c[:, 1:4, :])
    v.tensor_mul(qcd.rearrange("p (c x) -> p c x", c=3)[:, 0:2, :],
                 _bc(q4[:, f:2 * f], 2), q4c[:, 2:4, :])
    v.tensor_mul(qcd[:, 2 * f:3 * f], q4[:, 2 * f:3 * f], q4[:, 3 * f:4 * f])

    # ---- off-diagonal R entries (paired block ops) ------------------------
    # [C10|C21] = [bc|cd] + [ad|ab]
    v.tensor_add(_apv(r6, f, [[4 * f, 2], [1, f]]),
                 _apv(qcd, 0, [[2 * f, 2], [1, f]]),
                 _apv(pp, 2 * f, [[-2 * f, 2], [1, f]]))
    # [C20|C01] = [bd|bc] - [ac|ad]
    v.tensor_sub(_apv(r6, 2 * f, [[f, 2], [1, f]]),
                 _apv(qcd, f, [[-f, 2], [1, f]]),
                 _apv(pp, f, [[f, 2], [1, f]]))

    # ---- scales: s3 = exp(ls), tt = (sj - s2) / nu^2 ----------------------
    s3 = W(3 * f, tag="s3")
    s.activation(s3, ls3, AF.Exp)
    s2v = s3[:, 2 * f:3 * f]

    nusq = W(f, FP32, tag="nusq")
    s.activation(nusq, nu, AF.Square)
    iv = W(f, FP32, tag="iv")
    v.reciprocal_approx_fast(iv, nusq)
    ivh = W(f, FP16, tag="ivh")
    s.copy(out=ivh, in_=iv)

    tte = nc.gpsimd if GPS else v
    tt = st                                  # reuse [P, 2f] (st dead)
    ttj = tt.rearrange("p (j x) -> p j x", j=2)
    tte.tensor_sub(ttj,
                   s3.rearrange("p (j x) -> p j x", j=3)[:, 0:2, :],
                   _bc(s2v, 2))
    tte.tensor_mul(ttj, ttj, _bc(ivh, 2))

    # ---- Gram: cov6 = [c00|c11|c22|c01|c02|c12] ---------------------------
    # Off-diagonal chain first (independent of the ScalarE square of R6, so
    # the DVE overlaps it); diagonal chain after.
    sq6 = W(6 * f, tag="sq6")                # C entries squared (ScalarE)
    s.activation(sq6, r6, AF.Square)

    # v4 = [v00|v10|v01|v11] = t_j * (C0j, C1j)
    v4 = W(4 * f, tag="v4")
    r6j = r6.rearrange("p (j i x) -> p j i x", j=2, i=3)
    v.tensor_mul(v4.rearrange("p (j i x) -> p j i x", j=2, i=2),
                 r6j[:, :, 0:2, :],
                 ttj.unsqueeze(2).broadcast_to((P, 2, 2, f)))

    # pa = [v00*C10 | v00*C20 | v01*C11 | v01*C21]
    pa = sq4                                 # reuse [P, 4f] (sq4 dead)
    v4j = v4.rearrange("p (j i x) -> p j i x", j=2, i=2)
    v.tensor_mul(pa.rearrange("p (j i x) -> p j i x", j=2, i=2),
                 v4j[:, :, 0:1, :].broadcast_to((P, 2, 2, f)),
                 r6j[:, :, 1:3, :])
    # pb = [v10*C20 | v11*C21]
    pb = pq                                  # reuse [P, 2f] (pq dead)
    v.tensor_mul(pb.rearrange("p (j x) -> p j x", j=2),
                 v4j[:, :, 1:2, :].squeeze(2),
                 r6j[:, :, 2:3, :].squeeze(2))

    out6 = io.tile([P, 6 * f], FP16, tag="out6", name=f"out6_{t0}")
    # off-diagonals
    v.tensor_add(out6[:, 3 * f:5 * f], pa[:, 0:2 * f], pa[:, 2 * f:4 * f])
    v.tensor_add(out6[:, 5 * f:6 * f], pb[:, 0:f], pb[:, f:2 * f])

    # diagonal: wsq = sq6 * t_j (in place), then wsq[0:3]+wsq[3:6] + s2
    v.tensor_mul(sq6.rearrange("p (j i x) -> p j i x", j=2, i=3),
                 sq6.rearrange("p (j i x) -> p j i x", j=2, i=3),
                 ttj.unsqueeze(2).broadcast_to((P, 2, 3, f)))
    v.tensor_add(sq6[:, 0:3 * f], sq6[:, 0:3 * f], sq6[:, 3 * f:6 * f])
    (nc.gpsimd if GPS else v).tensor_add(
        out6.rearrange("p (e x) -> p e x", e=6)[:, 0:3, :],
        sq6.rearrange("p (e x) -> p e x", e=6)[:, 0:3, :],
        _bc(s2v, 3))

    nc.sync.dma_start(out=ov[:, :, t0:t0 + f],
                      in_=out6.rearrange("p (e x) -> p e x", e=6))


def _prep_inputs(quaternion, log_scale):
    n = quaternion.shape[0]
    pad = N_CORES * NPC - n
    if pad:
        qpad = np.tile(np.array([1, 0, 0, 0], np.float32), (pad, 1))
        lpad = np.zeros((pad, 3), np.float32)
        quaternion = np.concatenate([quaternion, qpad], axis=0)
        log_scale = np.concatenate([log_scale, lpad], axis=0)
    in_maps = []
    for i in range(N_CORES):
        sl = slice(i * NPC, (i + 1) * NPC)
        in_maps.append({
            "q": np.ascontiguousarray(quaternion[sl].T.astype(np.float16)),
            "ls": np.ascontiguousarray(log_scale[sl].T.astype(np.float16)),
        })
    return in_maps


def kernel_with_stats(quaternion, log_scale, trace=False):
    quaternion = np.asarray(quaternion, dtype=np.float32)
    log_scale = np.asarray(log_scale, dtype=np.float32)
    n = quaternion.shape[0]
    nc = _build()
    in_maps = _prep_inputs(quaternion, log_scale)
    res = run_bass_kernel_spmd(nc, in_maps, core_ids=list(range(N_CORES)), trace=trace)
    planes = np.concatenate([r["cov6"] for r in res.results], axis=1)[:, :n]
    planes = planes.astype(np.float32)
    out = np.empty((n, 3, 3), np.float32)
    out[:, 0, 0] = planes[0]
    out[:, 1, 1] = planes[1]
    out[:, 2, 2] = planes[2]
    out[:, 0, 1] = out[:, 1, 0] = planes[3]
    out[:, 0, 2] = out[:, 2, 0] = planes[4]
    out[:, 1, 2] = out[:, 2, 1] = planes[5]
    return out, res


def kernel(quaternion, log_scale):
    out, _ = kernel_with_stats(quaternion, log_scale, trace=False)
    return out


# revision 10
# speedup vs baseline: 1.2356x; 1.2356x over previous
"""Trainium2 Bass kernel: per-point 3x3 Gaussian covariance from quaternion + log_scale.

cov = R diag(exp(log_scale)) R^T with R from the normalized quaternion.

Identity used (avoids normalizing q and the third rotation column):
  nu  = |q|^2 / 2
  C   = nu * R   (entries are plain quadratics of raw q: C00 = ha+hb-hc-hd,
                  C10 = bc+ad, C20 = bd-ac, C01 = bc-ad, C11 = ha-hb+hc-hd,
                  C21 = cd+ab, with hx = x^2/2)
  cov = s2*I + t0'*C0 C0^T + t1'*C1 C1^T,  tj' = (sj - s2)/nu^2
(uses R R^T = I to eliminate column 2.)

Layout: host uploads planar fp16 q[4, NPC], ls[3, NPC]; output is the 6
unique covariance entries planar fp16 cov6[6, NPC]; host symmetrizes and
upcasts.  On-chip everything is fp16 contiguous/block APs so every DVE
tensor_tensor hits the 2x_1p mode; squares/exp/copy run on ScalarE (all in
the one `exp_and_others` table set -> single ACT_TABLE_LOAD).  1/nu^2 via
the custom DVE reciprocal_approx_fast (no Ln, no table switch).
"""

import os
import numpy as np

import concourse.bass as bass
import concourse.bacc as bacc
import concourse.mybir as mybir
from concourse.tile import TileContext
from concourse.bass_utils import run_bass_kernel_spmd

AF = mybir.ActivationFunctionType
FP32 = mybir.dt.float32
FP16 = mybir.dt.float16

N_CORES = 8
N_FULL = 4_000_000
P = 128

F = int(os.environ.get("KERNEL_F", "978"))      # points per partition per tile
GPS = int(os.environ.get("KERNEL_GPSIMD", "0"))  # offload small ops to GPSIMD
                                                 # (measured: SBUF-port contention
                                                 # slows DVE more than it saves)
NT = -(-3907 // F)                               # tiles so that P*R*8 >= N
R = F * NT                                       # rows per partition per core
NPC = P * R                                      # points per core (padded)

SQRT_HALF = 0.7071067811865476

_built = {}


def _apv(t, off, pairs):
    """Raw AP view of tile t: keep its partition dim, replace free dims.

    pairs = [[stride, count], ...] in elements, offset in elements from the
    tile's base.
    """
    ap = [list(p) for p in t.ap]
    return bass.AP(tensor=t.tensor, offset=t.offset + off, ap=[ap[0]] + pairs)


def _bc(ap2d, n):
    """[P, f] -> [P, n, f] broadcast (stride-0 middle dim)."""
    p, f = ap2d.shape
    return ap2d.unsqueeze(1).broadcast_to((p, n, f))


def _build():
    key = F
    if key in _built:
        return _built[key]

    nc = bacc.Bacc("TRN2", target_bir_lowering=False, debug=False, num_devices=N_CORES)
    q = nc.dram_tensor("q", [4, NPC], FP16, kind="ExternalInput")
    ls = nc.dram_tensor("ls", [3, NPC], FP16, kind="ExternalInput")
    cov = nc.dram_tensor("cov6", [6, NPC], FP16, kind="ExternalOutput")

    qv = q.ap().rearrange("c (p r) -> p c r", p=P)      # [P, 4, R]
    lsv = ls.ap().rearrange("c (p r) -> p c r", p=P)    # [P, 3, R]
    ov = cov.ap().rearrange("e (p r) -> p e r", p=P)    # [P, 6, R]

    with TileContext(nc) as tc:
        with (
            tc.tile_pool(name="io", bufs=2) as io,
            tc.tile_pool(name="wk", bufs=2) as wk,
        ):
            for it in range(NT):
                _tile_body(nc, io, wk, qv, lsv, ov, it * F, F)

    nc.compile()
    _built[key] = nc
    return nc


def _tile_body(nc, io, wk, qv, lsv, ov, t0, f):
    v = nc.vector
    s = nc.scalar

    def W(shape_f, dt=FP16, tag=None):
        return wk.tile([P, shape_f], dt, tag=tag, name=f"{tag}_{t0}")

    # ---- DMA in ----------------------------------------------------------
    q4 = io.tile([P, 4 * f], FP16, tag="q4", name=f"q4_{t0}")
    ls3 = io.tile([P, 3 * f], FP16, tag="ls3", name=f"ls3_{t0}")
    nc.sync.dma_start(out=q4.rearrange("p (c x) -> p c x", c=4),
                      in_=qv[:, :, t0:t0 + f])
    nc.sync.dma_start(out=ls3.rearrange("p (c x) -> p c x", c=3),
                      in_=lsv[:, :, t0:t0 + f])

    # ---- half-squares + nu + diagonal R entries --------------------------
    sq4 = W(4 * f, tag="sq4_pa")            # [ha|hb|hc|hd]
    s.activation(sq4, q4, AF.Square, scale=SQRT_HALF)

    pq = W(2 * f, tag="pq_pb")              # [ha-hc | hb-hd]
    st = W(2 * f, tag="st_tt")              # [ha+hc | hb+hd]
    v.tensor_sub(pq, sq4[:, 0:2 * f], sq4[:, 2 * f:4 * f])
    v.tensor_add(st, sq4[:, 0:2 * f], sq4[:, 2 * f:4 * f])

    r6 = W(6 * f, tag="r6")                 # [C00|C10|C20|C01|C11|C21]
    v.tensor_add(r6[:, 0:f], pq[:, 0:f], pq[:, f:2 * f])          # C00
    v.tensor_sub(r6[:, 4 * f:5 * f], pq[:, 0:f], pq[:, f:2 * f])  # C11
    nu = W(f, FP16, tag="nu")
    v.tensor_add(nu, st[:, 0:f], st[:, f:2 * f])

    # ---- raw quaternion products -----------------------------------------
    pp = W(3 * f, tag="pp")                 # [ab|ac|ad]
    qcd = W(3 * f, tag="qcd")               # [bc|bd|cd]
    q4c = q4.rearrange("p (c x) -> p c x", c=4)
    v.tensor_mul(pp.rearrange("p (c x) -> p c x", c=3),
                 _bc(q4[:, 0:f], 3), q4c[:, 1:4, :])
    v.tensor_mul(qcd.rearrange("p (c x) -> p c x", c=3)[:, 0:2, :],
                 _bc(q4[:, f:2 * f], 2), q4c[:, 2:4, :])
    v.tensor_mul(qcd[:, 2 * f:3 * f], q4[:, 2 * f:3 * f], q4[:, 3 * f:4 * f])

    # ---- off-diagonal R entries (paired block ops) ------------------------
    # [C10|C21] = [bc|cd] + [ad|ab]
    v.tensor_add(_apv(r6, f, [[4 * f, 2], [1, f]]),
                 _apv(qcd, 0, [[2 * f, 2], [1, f]]),
                 _apv(pp, 2 * f, [[-2 * f, 2], [1, f]]))
    # [C20|C01] = [bd|bc] - [ac|ad]
    v.tensor_sub(_apv(r6, 2 * f, [[f, 2], [1, f]]),
                 _apv(qcd, f, [[-f, 2], [1, f]]),
                 _apv(pp, f, [[f, 2], [1, f]]))

    # ---- scales: s3 = exp(ls), tt = (sj - s2) / nu^2 ----------------------
    s3 = W(3 * f, tag="s3")
    s.activation(s3, ls3, AF.Exp)
    s2v = s3[:, 2 * f:3 * f]

    nusq = W(f, FP32, tag="nusq")
    s.activation(nusq, nu, AF.Square)
    iv = W(f, FP32, tag="iv")
    v.reciprocal_approx_fast(iv, nusq)
    ivh = W(f, FP16, tag="ivh")
    s.copy(out=ivh, in_=iv)

    tte = nc.gpsimd if GPS else v
    tt = st                                  # reuse [P, 2f] (st dead)
    ttj = tt.rearrange("p (j x) -> p j x", j=2)
    tte.tensor_sub(ttj,
                   s3.rearrange("p (j x) -> p j x", j=3)[:, 0:2, :],
                   _bc(s2v, 2))
    tte.tensor_mul(ttj, ttj, _bc(ivh, 2))

    # ---- Gram: cov6 = [c00|c11|c22|c01|c02|c12] ---------------------------
    # Off-diagonal chain first (independent of the ScalarE square of R6, so
    # the DVE overlaps it); diagonal chain after.
    sq6 = W(6 * f, tag="sq6")                # C entries squared (ScalarE)
    s.activation(sq6, r6, AF.Square)

    # v4 = [v00|v10|v01|v11] = t_j * (C0j, C1j)
    v4 = W(4 * f, tag="v4")
    r6j = r6.rearrange("p (j i x) -> p j i x", j=2, i=3)
    v.tensor_mul(v4.rearrange("p (j i x) -> p j i x", j=2, i=2),
                 r6j[:, :, 0:2, :],
                 ttj.unsqueeze(2).broadcast_to((P, 2, 2, f)))

    # pa = [v00*C10 | v00*C20 | v01*C11 | v01*C21]
    pa = sq4                                 # reuse [P, 4f] (sq4 dead)
    v4j = v4.rearrange("p (j i x) -> p j i x", j=2, i=2)
    v.tensor_mul(pa.rearrange("p (j i x) -> p j i x", j=2, i=2),
                 v4j[:, :, 0:1, :].broadcast_to((P, 2, 2, f)),
                 r6j[:, :, 1:3, :])
    # pb = [v10*C20 | v11*C21]
    pb = pq                                  # reuse [P, 2f] (pq dead)
    v.tensor_mul(pb.rearrange("p (j x) -> p j x", j=2),
                 v4j[:, :, 1:2, :].squeeze(2),
                 r6j[:, :, 2:3, :].squeeze(2))

    out6 = io.tile([P, 6 * f], FP16, tag="out6", name=f"out6_{t0}")
    out6e = out6.rearrange("p (e x) -> p e x", e=6)
    # off-diagonals; stream their DMA out as soon as they are done
    v.tensor_add(out6[:, 3 * f:5 * f], pa[:, 0:2 * f], pa[:, 2 * f:4 * f])
    v.tensor_add(out6[:, 5 * f:6 * f], pb[:, 0:f], pb[:, f:2 * f])
    nc.sync.dma_start(out=ov[:, 3:6, t0:t0 + f], in_=out6e[:, 3:6, :])

    # diagonal: wsq = sq6 * t_j (in place), then wsq[0:3]+wsq[3:6] + s2
    v.tensor_mul(sq6.rearrange("p (j i x) -> p j i x", j=2, i=3),
                 sq6.rearrange("p (j i x) -> p j i x", j=2, i=3),
                 ttj.unsqueeze(2).broadcast_to((P, 2, 3, f)))
    v.tensor_add(sq6[:, 0:3 * f], sq6[:, 0:3 * f], sq6[:, 3 * f:6 * f])
    (nc.gpsimd if GPS else v).tensor_add(
        out6e[:, 0:3, :],
        sq6.rearrange("p (e x) -> p e x", e=6)[:, 0:3, :],
        _bc(s2v, 3))
    nc.sync.dma_start(out=ov[:, 0:3, t0:t0 + f], in_=out6e[:, 0:3, :])


def _prep_inputs(quaternion, log_scale):
    n = quaternion.shape[0]
    pad = N_CORES * NPC - n
    if pad:
        qpad = np.tile(np.array([1, 0, 0, 0], np.float32), (pad, 1))
        lpad = np.zeros((pad, 3), np.float32)
        quaternion = np.concatenate([quaternion, qpad], axis=0)
        log_scale = np.concatenate([log_scale, lpad], axis=0)
    in_maps = []
    for i in range(N_CORES):
        sl = slice(i * NPC, (i + 1) * NPC)
        in_maps.append({
            "q": np.ascontiguousarray(quaternion[sl].T.astype(np.float16)),
            "ls": np.ascontiguousarray(log_scale[sl].T.astype(np.float16)),
        })
    return in_maps


def kernel_with_stats(quaternion, log_scale, trace=False):
    quaternion = np.asarray(quaternion, dtype=np.float32)
    log_scale = np.asarray(log_scale, dtype=np.float32)
    n = quaternion.shape[0]
    nc = _build()
    in_maps = _prep_inputs(quaternion, log_scale)
    res = run_bass_kernel_spmd(nc, in_maps, core_ids=list(range(N_CORES)), trace=trace)
    planes = np.concatenate([r["cov6"] for r in res.results], axis=1)[:, :n]
    planes = planes.astype(np.float32)
    out = np.empty((n, 3, 3), np.float32)
    out[:, 0, 0] = planes[0]
    out[:, 1, 1] = planes[1]
    out[:, 2, 2] = planes[2]
    out[:, 0, 1] = out[:, 1, 0] = planes[3]
    out[:, 0, 2] = out[:, 2, 0] = planes[4]
    out[:, 1, 2] = out[:, 2, 1] = planes[5]
    return out, res


def kernel(quaternion, log_scale):
    out, _ = kernel_with_stats(quaternion, log_scale, trace=False)
    return out


# revision 13
# speedup vs baseline: 1.2485x; 1.0105x over previous
"""Trainium2 Bass kernel: per-point 3x3 Gaussian covariance from quaternion + log_scale.

cov = R diag(exp(log_scale)) R^T with R from the normalized quaternion.

Identity used (avoids normalizing q and the third rotation column):
  nu  = |q|^2 / 2
  C   = nu * R   (entries are plain quadratics of raw q: C00 = ha+hb-hc-hd,
                  C10 = bc+ad, C20 = bd-ac, C01 = bc-ad, C11 = ha-hb+hc-hd,
                  C21 = cd+ab, with hx = x^2/2)
  cov = s2*I + t0'*C0 C0^T + t1'*C1 C1^T,  tj' = (sj - s2)/nu^2
(uses R R^T = I to eliminate column 2.)

Layout: host uploads planar fp16 q[4, NPC], ls[3, NPC]; output is the 6
unique covariance entries planar fp16 cov6[6, NPC]; host symmetrizes and
upcasts.  On-chip everything is fp16 contiguous/block APs so every DVE
tensor_tensor hits the 2x_1p mode; squares/exp/copy run on ScalarE (all in
the one `exp_and_others` table set -> single ACT_TABLE_LOAD).  1/nu^2 via
the custom DVE reciprocal_approx_fast (no Ln, no table switch).
"""

import os
import numpy as np

import concourse.bass as bass
import concourse.bacc as bacc
import concourse.mybir as mybir
from concourse.tile import TileContext
from concourse.bass_utils import run_bass_kernel_spmd

AF = mybir.ActivationFunctionType
FP32 = mybir.dt.float32
FP16 = mybir.dt.float16

N_CORES = 8
N_FULL = 4_000_000
P = 128

F = int(os.environ.get("KERNEL_F", "978"))      # points per partition per tile
GPS = int(os.environ.get("KERNEL_GPSIMD", "0"))  # offload small ops to GPSIMD
                                                 # (measured: SBUF-port contention
                                                 # slows DVE more than it saves)
NT = -(-3907 // F)                               # tiles so that P*R*8 >= N
R = F * NT                                       # rows per partition per core
NPC = P * R                                      # points per core (padded)

SQRT_HALF = 0.7071067811865476

_built = {}


def _apv(t, off, pairs):
    """Raw AP view of tile t: keep its partition dim, replace free dims.

    pairs = [[stride, count], ...] in elements, offset in elements from the
    tile's base.
    """
    ap = [list(p) for p in t.ap]
    return bass.AP(tensor=t.tensor, offset=t.offset + off, ap=[ap[0]] + pairs)


def _bc(ap2d, n):
    """[P, f] -> [P, n, f] broadcast (stride-0 middle dim)."""
    p, f = ap2d.shape
    return ap2d.unsqueeze(1).broadcast_to((p, n, f))


def _build():
    key = F
    if key in _built:
        return _built[key]

    nc = bacc.Bacc("TRN2", target_bir_lowering=False, debug=False, num_devices=N_CORES)
    q = nc.dram_tensor("q", [4, NPC], FP16, kind="ExternalInput")
    ls = nc.dram_tensor("ls", [3, NPC], FP16, kind="ExternalInput")
    cov = nc.dram_tensor("cov6", [6, NPC], FP16, kind="ExternalOutput")

    qv = q.ap().rearrange("c (p r) -> p c r", p=P)      # [P, 4, R]
    lsv = ls.ap().rearrange("c (p r) -> p c r", p=P)    # [P, 3, R]
    ov = cov.ap().rearrange("e (p r) -> p e r", p=P)    # [P, 6, R]

    with TileContext(nc) as tc:
        with (
            tc.tile_pool(name="io", bufs=2) as io,
            tc.tile_pool(name="wk", bufs=2) as wk,
        ):
            for it in range(NT):
                _tile_body(nc, io, wk, qv, lsv, ov, it * F, F)

    nc.compile()
    _built[key] = nc
    return nc


def _tile_body(nc, io, wk, qv, lsv, ov, t0, f):
    v = nc.vector
    s = nc.scalar

    def W(shape_f, dt=FP16, tag=None):
        return wk.tile([P, shape_f], dt, tag=tag, name=f"{tag}_{t0}")

    # ---- DMA in ----------------------------------------------------------
    q4 = io.tile([P, 4 * f], FP16, tag="q4", name=f"q4_{t0}")
    ls3 = io.tile([P, 3 * f], FP16, tag="ls3", name=f"ls3_{t0}")
    nc.sync.dma_start(out=q4.rearrange("p (c x) -> p c x", c=4),
                      in_=qv[:, :, t0:t0 + f])
    nc.sync.dma_start(out=ls3.rearrange("p (c x) -> p c x", c=3),
                      in_=lsv[:, :, t0:t0 + f])

    # ---- half-squares + nu + diagonal R entries --------------------------
    sq4 = W(4 * f, tag="sq4_pa")            # [ha|hb|hc|hd]
    s.activation(sq4, q4, AF.Square, scale=SQRT_HALF)

    pq = W(2 * f, tag="pq_pb")              # [ha-hc | hb-hd]
    st = W(2 * f, tag="st_tt")              # [ha+hc | hb+hd]
    v.tensor_sub(pq, sq4[:, 0:2 * f], sq4[:, 2 * f:4 * f])
    v.tensor_add(st, sq4[:, 0:2 * f], sq4[:, 2 * f:4 * f])

    r6 = W(6 * f, tag="r6")                 # [C00|C10|C20|C01|C11|C21]
    v.tensor_add(r6[:, 0:f], pq[:, 0:f], pq[:, f:2 * f])          # C00
    v.tensor_sub(r6[:, 4 * f:5 * f], pq[:, 0:f], pq[:, f:2 * f])  # C11
    nu = W(f, FP16, tag="nu")
    v.tensor_add(nu, st[:, 0:f], st[:, f:2 * f])

    # ---- raw quaternion products -----------------------------------------
    pp = W(3 * f, tag="pp")                 # [ab|ac|ad]
    qcd = W(3 * f, tag="qcd")               # [bc|bd|cd]
    q4c = q4.rearrange("p (c x) -> p c x", c=4)
    v.tensor_mul(pp.rearrange("p (c x) -> p c x", c=3),
                 _bc(q4[:, 0:f], 3), q4c[:, 1:4, :])
    v.tensor_mul(qcd.rearrange("p (c x) -> p c x", c=3)[:, 0:2, :],
                 _bc(q4[:, f:2 * f], 2), q4c[:, 2:4, :])
    v.tensor_mul(qcd[:, 2 * f:3 * f], q4[:, 2 * f:3 * f], q4[:, 3 * f:4 * f])

    # ---- off-diagonal R entries (paired block ops) ------------------------
    # [C10|C21] = [bc|cd] + [ad|ab]
    v.tensor_add(_apv(r6, f, [[4 * f, 2], [1, f]]),
                 _apv(qcd, 0, [[2 * f, 2], [1, f]]),
                 _apv(pp, 2 * f, [[-2 * f, 2], [1, f]]))
    # [C20|C01] = [bd|bc] - [ac|ad]
    v.tensor_sub(_apv(r6, 2 * f, [[f, 2], [1, f]]),
                 _apv(qcd, f, [[-f, 2], [1, f]]),
                 _apv(pp, f, [[f, 2], [1, f]]))

    # ---- scales: s3 = exp(ls), tt = (sj - s2) / nu^2 ----------------------
    s3 = W(3 * f, tag="s3")
    s.activation(s3, ls3, AF.Exp)
    s2v = s3[:, 2 * f:3 * f]

    nusq = W(f, FP32, tag="nusq")
    s.activation(nusq, nu, AF.Square)
    iv = W(f, FP32, tag="iv")
    v.reciprocal_approx_fast(iv, nusq)
    ivh = W(f, FP16, tag="ivh")
    s.copy(out=ivh, in_=iv)

    tte = nc.gpsimd if GPS else v
    tt = st                                  # reuse [P, 2f] (st dead)
    ttj = tt.rearrange("p (j x) -> p j x", j=2)
    tte.tensor_sub(ttj,
                   s3.rearrange("p (j x) -> p j x", j=3)[:, 0:2, :],
                   _bc(s2v, 2))
    tte.tensor_mul(ttj, ttj, _bc(ivh, 2))

    # ---- Gram: cov6 = [c00|c11|c22|c01|c02|c12] ---------------------------
    # Off-diagonal chain first (independent of the ScalarE square of R6, so
    # the DVE overlaps it); diagonal chain after.
    sq6 = W(6 * f, tag="sq6")                # C entries squared (ScalarE)
    s.activation(sq6[:, 0:3 * f], r6[:, 0:3 * f], AF.Square)
    s.activation(sq6[:, 3 * f:6 * f], r6[:, 3 * f:6 * f], AF.Square)

    # v4 = [v00|v10|v01|v11] = t_j * (C0j, C1j)
    v4 = W(4 * f, tag="v4")
    r6j = r6.rearrange("p (j i x) -> p j i x", j=2, i=3)
    v.tensor_mul(v4.rearrange("p (j i x) -> p j i x", j=2, i=2),
                 r6j[:, :, 0:2, :],
                 ttj.unsqueeze(2).broadcast_to((P, 2, 2, f)))

    # pa = [v00*C10 | v00*C20 | v01*C11 | v01*C21]
    pa = sq4                                 # reuse [P, 4f] (sq4 dead)
    v4j = v4.rearrange("p (j i x) -> p j i x", j=2, i=2)
    v.tensor_mul(pa.rearrange("p (j i x) -> p j i x", j=2, i=2),
                 v4j[:, :, 0:1, :].broadcast_to((P, 2, 2, f)),
                 r6j[:, :, 1:3, :])
    # pb = [v10*C20 | v11*C21]
    pb = pq                                  # reuse [P, 2f] (pq dead)
    v.tensor_mul(pb.rearrange("p (j x) -> p j x", j=2),
                 v4j[:, :, 1:2, :].squeeze(2),
                 r6j[:, :, 2:3, :].squeeze(2))

    out6 = io.tile([P, 6 * f], FP16, tag="out6", name=f"out6_{t0}")
    out6e = out6.rearrange("p (e x) -> p e x", e=6)
    # off-diagonals; stream their DMA out as soon as they are done
    v.tensor_add(out6[:, 3 * f:5 * f], pa[:, 0:2 * f], pa[:, 2 * f:4 * f])
    v.tensor_add(out6[:, 5 * f:6 * f], pb[:, 0:f], pb[:, f:2 * f])
    nc.sync.dma_start(out=ov[:, 3:6, t0:t0 + f], in_=out6e[:, 3:6, :])

    # diagonal: wsq = sq6 * t_j (in place, split by column j), then
    # wsq[0:3]+wsq[3:6] + s2
    sq6e = sq6.rearrange("p (e x) -> p e x", e=6)
    v.tensor_mul(sq6e[:, 0:3, :], sq6e[:, 0:3, :], _bc(tt[:, 0:f], 3))
    v.tensor_mul(sq6e[:, 3:6, :], sq6e[:, 3:6, :], _bc(tt[:, f:2 * f], 3))
    v.tensor_add(sq6[:, 0:3 * f], sq6[:, 0:3 * f], sq6[:, 3 * f:6 * f])
    (nc.gpsimd if GPS else v).tensor_add(
        out6e[:, 0:3, :],
        sq6.rearrange("p (e x) -> p e x", e=6)[:, 0:3, :],
        _bc(s2v, 3))
    nc.sync.dma_start(out=ov[:, 0:3, t0:t0 + f], in_=out6e[:, 0:3, :])


def _prep_inputs(quaternion, log_scale):
    n = quaternion.shape[0]
    pad = N_CORES * NPC - n
    if pad:
        qpad = np.tile(np.array([1, 0, 0, 0], np.float32), (pad, 1))
        lpad = np.zeros((pad, 3), np.float32)
        quaternion = np.concatenate([quaternion, qpad], axis=0)
        log_scale = np.concatenate([log_scale, lpad], axis=0)
    in_maps = []
    for i in range(N_CORES):
        sl = slice(i * NPC, (i + 1) * NPC)
        in_maps.append({
            "q": np.ascontiguousarray(quaternion[sl].T.astype(np.float16)),
            "ls": np.ascontiguousarray(log_scale[sl].T.astype(np.float16)),
        })
    return in_maps


def kernel_with_stats(quaternion, log_scale, trace=False):
    quaternion = np.asarray(quaternion, dtype=np.float32)
    log_scale = np.asarray(log_scale, dtype=np.float32)
    n = quaternion.shape[0]
    nc = _build()
    in_maps = _prep_inputs(quaternion, log_scale)
    res = run_bass_kernel_spmd(nc, in_maps, core_ids=list(range(N_CORES)), trace=trace)
    planes = np.concatenate([r["cov6"] for r in res.results], axis=1)[:, :n]
    planes = planes.astype(np.float32)
    out = np.empty((n, 3, 3), np.float32)
    out[:, 0, 0] = planes[0]
    out[:, 1, 1] = planes[1]
    out[:, 2, 2] = planes[2]
    out[:, 0, 1] = out[:, 1, 0] = planes[3]
    out[:, 0, 2] = out[:, 2, 0] = planes[4]
    out[:, 1, 2] = out[:, 2, 1] = planes[5]
    return out, res


def kernel(quaternion, log_scale):
    out, _ = kernel_with_stats(quaternion, log_scale, trace=False)
    return out


# revision 14
# speedup vs baseline: 1.2534x; 1.0040x over previous
"""Trainium2 Bass kernel: per-point 3x3 Gaussian covariance from quaternion + log_scale.

cov = R diag(exp(log_scale)) R^T with R from the normalized quaternion.

Identity used (avoids normalizing q and the third rotation column):
  nu  = |q|^2 / 2
  C   = nu * R   (entries are plain quadratics of raw q: C00 = ha+hb-hc-hd,
                  C10 = bc+ad, C20 = bd-ac, C01 = bc-ad, C11 = ha-hb+hc-hd,
                  C21 = cd+ab, with hx = x^2/2)
  cov = s2*I + t0'*C0 C0^T + t1'*C1 C1^T,  tj' = (sj - s2)/nu^2
(uses R R^T = I to eliminate column 2.)

Layout: host uploads planar fp16 q[4, NPC], ls[3, NPC]; output is the 6
unique covariance entries planar fp16 cov6[6, NPC]; host symmetrizes and
upcasts.  On-chip everything is fp16 contiguous/block APs so every DVE
tensor_tensor hits the 2x_1p mode; squares/exp/copy run on ScalarE (all in
the one `exp_and_others` table set -> single ACT_TABLE_LOAD).  1/nu^2 via
the custom DVE reciprocal_approx_fast (no Ln, no table switch).
"""

import os
import numpy as np

import concourse.bass as bass
import concourse.bacc as bacc
import concourse.mybir as mybir
from concourse.tile import TileContext
from concourse.bass_utils import run_bass_kernel_spmd

AF = mybir.ActivationFunctionType
FP32 = mybir.dt.float32
FP16 = mybir.dt.float16

N_CORES = 8
N_FULL = 4_000_000
P = 128

F = int(os.environ.get("KERNEL_F", "978"))      # points per partition per tile
GPS = int(os.environ.get("KERNEL_GPSIMD", "0"))  # offload small ops to GPSIMD
                                                 # (measured: SBUF-port contention
                                                 # slows DVE more than it saves)
NT = -(-3907 // F)                               # tiles so that P*R*8 >= N
R = F * NT                                       # rows per partition per core
NPC = P * R                                      # points per core (padded)

SQRT_HALF = 0.7071067811865476

_built = {}


def _apv(t, off, pairs):
    """Raw AP view of tile t: keep its partition dim, replace free dims.

    pairs = [[stride, count], ...] in elements, offset in elements from the
    tile's base.
    """
    ap = [list(p) for p in t.ap]
    return bass.AP(tensor=t.tensor, offset=t.offset + off, ap=[ap[0]] + pairs)


def _bc(ap2d, n):
    """[P, f] -> [P, n, f] broadcast (stride-0 middle dim)."""
    p, f = ap2d.shape
    return ap2d.unsqueeze(1).broadcast_to((p, n, f))


def _build():
    key = F
    if key in _built:
        return _built[key]

    nc = bacc.Bacc("TRN2", target_bir_lowering=False, debug=False, num_devices=N_CORES)
    q = nc.dram_tensor("q", [4, NPC], FP16, kind="ExternalInput")
    ls = nc.dram_tensor("ls", [3, NPC], FP16, kind="ExternalInput")
    cov = nc.dram_tensor("cov6", [6, NPC], FP16, kind="ExternalOutput")

    qv = q.ap().rearrange("c (p r) -> p c r", p=P)      # [P, 4, R]
    lsv = ls.ap().rearrange("c (p r) -> p c r", p=P)    # [P, 3, R]
    ov = cov.ap().rearrange("e (p r) -> p e r", p=P)    # [P, 6, R]

    with TileContext(nc) as tc:
        with (
            tc.tile_pool(name="io", bufs=2) as io,
            tc.tile_pool(name="wk", bufs=2) as wk,
        ):
            for it in range(NT):
                _tile_body(nc, io, wk, qv, lsv, ov, it * F, F)

    nc.compile()
    _built[key] = nc
    return nc


def _tile_body(nc, io, wk, qv, lsv, ov, t0, f):
    v = nc.vector
    s = nc.scalar

    def W(shape_f, dt=FP16, tag=None):
        return wk.tile([P, shape_f], dt, tag=tag, name=f"{tag}_{t0}")

    # ---- DMA in (q4 first; ls3 is only needed by the Exp much later) -----
    q4 = io.tile([P, 4 * f], FP16, tag="q4", name=f"q4_{t0}")
    ls3 = io.tile([P, 3 * f], FP16, tag="ls3", name=f"ls3_{t0}")
    nc.sync.dma_start(out=q4.rearrange("p (c x) -> p c x", c=4),
                      in_=qv[:, :, t0:t0 + f])

    # ---- half-squares + nu + diagonal R entries --------------------------
    sq4 = W(4 * f, tag="sq4_pa")            # [ha|hb|hc|hd]
    s.activation(sq4, q4, AF.Square, scale=SQRT_HALF)

    nc.sync.dma_start(out=ls3.rearrange("p (c x) -> p c x", c=3),
                      in_=lsv[:, :, t0:t0 + f])

    pq = W(2 * f, tag="pq_pb")              # [ha-hc | hb-hd]
    st = W(2 * f, tag="st_tt")              # [ha+hc | hb+hd]
    v.tensor_sub(pq, sq4[:, 0:2 * f], sq4[:, 2 * f:4 * f])
    v.tensor_add(st, sq4[:, 0:2 * f], sq4[:, 2 * f:4 * f])

    r6 = W(6 * f, tag="r6")                 # [C00|C10|C20|C01|C11|C21]
    v.tensor_add(r6[:, 0:f], pq[:, 0:f], pq[:, f:2 * f])          # C00
    v.tensor_sub(r6[:, 4 * f:5 * f], pq[:, 0:f], pq[:, f:2 * f])  # C11
    nu = W(f, FP16, tag="nu")
    v.tensor_add(nu, st[:, 0:f], st[:, f:2 * f])

    # ---- raw quaternion products -----------------------------------------
    pp = W(3 * f, tag="pp")                 # [ab|ac|ad]
    qcd = W(3 * f, tag="qcd")               # [bc|bd|cd]
    q4c = q4.rearrange("p (c x) -> p c x", c=4)
    v.tensor_mul(pp.rearrange("p (c x) -> p c x", c=3),
                 _bc(q4[:, 0:f], 3), q4c[:, 1:4, :])
    v.tensor_mul(qcd.rearrange("p (c x) -> p c x", c=3)[:, 0:2, :],
                 _bc(q4[:, f:2 * f], 2), q4c[:, 2:4, :])
    v.tensor_mul(qcd[:, 2 * f:3 * f], q4[:, 2 * f:3 * f], q4[:, 3 * f:4 * f])

    # ---- off-diagonal R entries (paired block ops) ------------------------
    # [C10|C21] = [bc|cd] + [ad|ab]
    v.tensor_add(_apv(r6, f, [[4 * f, 2], [1, f]]),
                 _apv(qcd, 0, [[2 * f, 2], [1, f]]),
                 _apv(pp, 2 * f, [[-2 * f, 2], [1, f]]))
    # [C20|C01] = [bd|bc] - [ac|ad]
    v.tensor_sub(_apv(r6, 2 * f, [[f, 2], [1, f]]),
                 _apv(qcd, f, [[-f, 2], [1, f]]),
                 _apv(pp, f, [[f, 2], [1, f]]))

    # ---- scales: s3 = exp(ls), tt = (sj - s2) / nu^2 ----------------------
    s3 = W(3 * f, tag="s3")
    s.activation(s3, ls3, AF.Exp)
    s2v = s3[:, 2 * f:3 * f]

    nusq = W(f, FP32, tag="nusq")
    s.activation(nusq, nu, AF.Square)
    iv = W(f, FP32, tag="iv")
    v.reciprocal_approx_fast(iv, nusq)
    ivh = W(f, FP16, tag="ivh")
    s.copy(out=ivh, in_=iv)

    tte = nc.gpsimd if GPS else v
    tt = st                                  # reuse [P, 2f] (st dead)
    ttj = tt.rearrange("p (j x) -> p j x", j=2)
    tte.tensor_sub(ttj,
                   s3.rearrange("p (j x) -> p j x", j=3)[:, 0:2, :],
                   _bc(s2v, 2))
    tte.tensor_mul(ttj, ttj, _bc(ivh, 2))

    # ---- Gram: cov6 = [c00|c11|c22|c01|c02|c12] ---------------------------
    # Off-diagonal chain first (independent of the ScalarE square of R6, so
    # the DVE overlaps it); diagonal chain after.
    sq6 = W(6 * f, tag="sq6")                # C entries squared (ScalarE)
    s.activation(sq6[:, 0:3 * f], r6[:, 0:3 * f], AF.Square)
    s.activation(sq6[:, 3 * f:6 * f], r6[:, 3 * f:6 * f], AF.Square)

    # v4 = [v00|v10|v01|v11] = t_j * (C0j, C1j)
    v4 = W(4 * f, tag="v4")
    r6j = r6.rearrange("p (j i x) -> p j i x", j=2, i=3)
    v.tensor_mul(v4.rearrange("p (j i x) -> p j i x", j=2, i=2),
                 r6j[:, :, 0:2, :],
                 ttj.unsqueeze(2).broadcast_to((P, 2, 2, f)))

    # pa = [v00*C10 | v00*C20 | v01*C11 | v01*C21]
    pa = sq4                                 # reuse [P, 4f] (sq4 dead)
    v4j = v4.rearrange("p (j i x) -> p j i x", j=2, i=2)
    v.tensor_mul(pa.rearrange("p (j i x) -> p j i x", j=2, i=2),
                 v4j[:, :, 0:1, :].broadcast_to((P, 2, 2, f)),
                 r6j[:, :, 1:3, :])
    # pb = [v10*C20 | v11*C21]
    pb = pq                                  # reuse [P, 2f] (pq dead)
    v.tensor_mul(pb.rearrange("p (j x) -> p j x", j=2),
                 v4j[:, :, 1:2, :].squeeze(2),
                 r6j[:, :, 2:3, :].squeeze(2))

    out6 = io.tile([P, 6 * f], FP16, tag="out6", name=f"out6_{t0}")
    out6e = out6.rearrange("p (e x) -> p e x", e=6)
    # off-diagonals; stream their DMA out as soon as they are done
    v.tensor_add(out6[:, 3 * f:5 * f], pa[:, 0:2 * f], pa[:, 2 * f:4 * f])
    v.tensor_add(out6[:, 5 * f:6 * f], pb[:, 0:f], pb[:, f:2 * f])
    nc.sync.dma_start(out=ov[:, 3:6, t0:t0 + f], in_=out6e[:, 3:6, :])

    # diagonal: wsq = sq6 * t_j (in place, split by column j), then
    # wsq[0:3]+wsq[3:6] + s2
    sq6e = sq6.rearrange("p (e x) -> p e x", e=6)
    v.tensor_mul(sq6e[:, 0:3, :], sq6e[:, 0:3, :], _bc(tt[:, 0:f], 3))
    v.tensor_mul(sq6e[:, 3:6, :], sq6e[:, 3:6, :], _bc(tt[:, f:2 * f], 3))
    v.tensor_add(sq6[:, 0:3 * f], sq6[:, 0:3 * f], sq6[:, 3 * f:6 * f])
    (nc.gpsimd if GPS else v).tensor_add(
        out6e[:, 0:3, :],
        sq6.rearrange("p (e x) -> p e x", e=6)[:, 0:3, :],
        _bc(s2v, 3))
    nc.sync.dma_start(out=ov[:, 0:3, t0:t0 + f], in_=out6e[:, 0:3, :])


def _prep_inputs(quaternion, log_scale):
    n = quaternion.shape[0]
    pad = N_CORES * NPC - n
    if pad:
        qpad = np.tile(np.array([1, 0, 0, 0], np.float32), (pad, 1))
        lpad = np.zeros((pad, 3), np.float32)
        quaternion = np.concatenate([quaternion, qpad], axis=0)
        log_scale = np.concatenate([log_scale, lpad], axis=0)
    in_maps = []
    for i in range(N_CORES):
        sl = slice(i * NPC, (i + 1) * NPC)
        in_maps.append({
            "q": np.ascontiguousarray(quaternion[sl].T.astype(np.float16)),
            "ls": np.ascontiguousarray(log_scale[sl].T.astype(np.float16)),
        })
    return in_maps


def kernel_with_stats(quaternion, log_scale, trace=False):
    quaternion = np.asarray(quaternion, dtype=np.float32)
    log_scale = np.asarray(log_scale, dtype=np.float32)
    n = quaternion.shape[0]
    nc = _build()
    in_maps = _prep_inputs(quaternion, log_scale)
    res = run_bass_kernel_spmd(nc, in_maps, core_ids=list(range(N_CORES)), trace=trace)
    planes = np.concatenate([r["cov6"] for r in res.results], axis=1)[:, :n]
    planes = planes.astype(np.float32)
    out = np.empty((n, 3, 3), np.float32)
    out[:, 0, 0] = planes[0]
    out[:, 1, 1] = planes[1]
    out[:, 2, 2] = planes[2]
    out[:, 0, 1] = out[:, 1, 0] = planes[3]
    out[:, 0, 2] = out[:, 2, 0] = planes[4]
    out[:, 1, 2] = out[:, 2, 1] = planes[5]
    return out, res


def kernel(quaternion, log_scale):
    out, _ = kernel_with_stats(quaternion, log_scale, trace=False)
    return out


# revision 22
# speedup vs baseline: 1.2703x; 1.0135x over previous
"""Trainium2 Bass kernel: per-point 3x3 Gaussian covariance from quaternion + log_scale.

cov = R diag(exp(log_scale)) R^T with R from the normalized quaternion.

Identity used (avoids normalizing q and the third rotation column):
  nu  = |q|^2 / 2
  C   = nu * R   (entries are plain quadratics of raw q: C00 = ha+hb-hc-hd,
                  C10 = bc+ad, C20 = bd-ac, C01 = bc-ad, C11 = ha-hb+hc-hd,
                  C21 = cd+ab, with hx = x^2/2)
  cov = s2*I + t0'*C0 C0^T + t1'*C1 C1^T,  tj' = (sj - s2)/nu^2
(uses R R^T = I to eliminate column 2.)

Layout: host uploads planar fp16 q[4, NPC], ls[3, NPC]; output is the 6
unique covariance entries planar fp16 cov6[6, NPC]; host symmetrizes and
upcasts.  On-chip everything is fp16 contiguous/block APs so every DVE
tensor_tensor hits the 2x_1p mode; squares/exp/copy run on ScalarE (all in
the one `exp_and_others` table set -> single ACT_TABLE_LOAD).  1/nu^2 via
the custom DVE reciprocal_approx_fast (no Ln, no table switch).
"""

import os
import numpy as np

import concourse.bass as bass
import concourse.bacc as bacc
import concourse.mybir as mybir
from concourse.tile import TileContext
from concourse.bass_utils import run_bass_kernel_spmd

AF = mybir.ActivationFunctionType
FP32 = mybir.dt.float32
FP16 = mybir.dt.float16

N_CORES = 8
N_FULL = 4_000_000
P = 128

F = int(os.environ.get("KERNEL_F", "978"))      # points per partition per tile
GPS = int(os.environ.get("KERNEL_GPSIMD", "0"))  # offload small ops to GPSIMD
                                                 # (measured: SBUF-port contention
                                                 # slows DVE more than it saves)
NT = -(-3907 // F)                               # tiles so that P*R*8 >= N
R = F * NT                                       # rows per partition per core
NPC = P * R                                      # points per core (padded)

SQRT_HALF = 0.7071067811865476

_built = {}


def _apv(t, off, pairs):
    """Raw AP view of tile t: keep its partition dim, replace free dims.

    pairs = [[stride, count], ...] in elements, offset in elements from the
    tile's base.
    """
    ap = [list(p) for p in t.ap]
    return bass.AP(tensor=t.tensor, offset=t.offset + off, ap=[ap[0]] + pairs)


def _bc(ap2d, n):
    """[P, f] -> [P, n, f] broadcast (stride-0 middle dim)."""
    p, f = ap2d.shape
    return ap2d.unsqueeze(1).broadcast_to((p, n, f))


def _build():
    key = F
    if key in _built:
        return _built[key]

    nc = bacc.Bacc("TRN2", target_bir_lowering=False, debug=False, num_devices=N_CORES)
    # Tile-major layouts: per (tile, partition) the data is contiguous in
    # DRAM, so every DMA descriptor is one large chunk per partition.
    q = nc.dram_tensor("q", [NT, P, 4, F], FP16, kind="ExternalInput")
    ls = nc.dram_tensor("ls", [NT, P, 3, F], FP16, kind="ExternalInput")
    cov = nc.dram_tensor("cov6", [NT, P, 6, F], FP16, kind="ExternalOutput")

    qv = q.ap()
    lsv = ls.ap()
    ov = cov.ap()

    with TileContext(nc) as tc:
        with (
            tc.tile_pool(name="io", bufs=2) as io,
            tc.tile_pool(name="wk", bufs=2) as wk,
        ):
            for it in range(NT):
                _tile_body(nc, io, wk, qv[it], lsv[it], ov[it], it * F, F)

    nc.compile()
    _built[key] = nc
    return nc


def _tile_body(nc, io, wk, qv, lsv, ov, t0, f):
    v = nc.vector
    s = nc.scalar

    def W(shape_f, dt=FP16, tag=None):
        return wk.tile([P, shape_f], dt, tag=tag, name=f"{tag}_{t0}")

    # ---- DMA in (q4 first; ls3 is only needed by the Exp much later) -----
    q4 = io.tile([P, 4 * f], FP16, tag="q4", name=f"q4_{t0}")
    ls3 = io.tile([P, 3 * f], FP16, tag="ls3", name=f"ls3_{t0}")
    nc.sync.dma_start(out=q4.rearrange("p (c x) -> p c x", c=4), in_=qv)

    # ---- half-squares + nu + diagonal R entries --------------------------
    sq4 = W(4 * f, tag="sq4_pa")            # [ha|hb|hc|hd]
    s.activation(sq4, q4, AF.Square, scale=SQRT_HALF)

    nc.sync.dma_start(out=ls3.rearrange("p (c x) -> p c x", c=3), in_=lsv)

    pq = W(2 * f, tag="pq_pb")              # [ha-hc | hb-hd]
    st = W(2 * f, tag="st_tt")              # [ha+hc | hb+hd]
    v.tensor_sub(pq, sq4[:, 0:2 * f], sq4[:, 2 * f:4 * f])
    v.tensor_add(st, sq4[:, 0:2 * f], sq4[:, 2 * f:4 * f])

    r6 = W(6 * f, tag="r6")                 # [C00|C10|C20|C01|C11|C21]
    v.tensor_add(r6[:, 0:f], pq[:, 0:f], pq[:, f:2 * f])          # C00
    v.tensor_sub(r6[:, 4 * f:5 * f], pq[:, 0:f], pq[:, f:2 * f])  # C11
    nu = W(f, FP16, tag="nu")
    v.tensor_add(nu, st[:, 0:f], st[:, f:2 * f])

    # ---- raw quaternion products -----------------------------------------
    pp = W(3 * f, tag="pp")                 # [ab|ac|ad]
    qcd = W(3 * f, tag="qcd")               # [bc|bd|cd]
    q4c = q4.rearrange("p (c x) -> p c x", c=4)
    v.tensor_mul(pp.rearrange("p (c x) -> p c x", c=3),
                 _bc(q4[:, 0:f], 3), q4c[:, 1:4, :])
    v.tensor_mul(qcd.rearrange("p (c x) -> p c x", c=3)[:, 0:2, :],
                 _bc(q4[:, f:2 * f], 2), q4c[:, 2:4, :])
    v.tensor_mul(qcd[:, 2 * f:3 * f], q4[:, 2 * f:3 * f], q4[:, 3 * f:4 * f])

    # ---- off-diagonal R entries (paired block ops) ------------------------
    # [C10|C21] = [bc|cd] + [ad|ab]
    v.tensor_add(_apv(r6, f, [[4 * f, 2], [1, f]]),
                 _apv(qcd, 0, [[2 * f, 2], [1, f]]),
                 _apv(pp, 2 * f, [[-2 * f, 2], [1, f]]))
    # [C20|C01] = [bd|bc] - [ac|ad]
    v.tensor_sub(_apv(r6, 2 * f, [[f, 2], [1, f]]),
                 _apv(qcd, f, [[-f, 2], [1, f]]),
                 _apv(pp, f, [[f, 2], [1, f]]))

    # ---- scales: s3 = exp(ls), tt = (sj - s2) / nu^2 ----------------------
    s3 = W(3 * f, tag="s3")
    s.activation(s3, ls3, AF.Exp)
    s2v = s3[:, 2 * f:3 * f]

    nusq = W(f, FP32, tag="nusq")
    s.activation(nusq, nu, AF.Square)
    iv = W(f, FP32, tag="iv")
    v.reciprocal_approx_fast(iv, nusq)
    ivh = W(f, FP16, tag="ivh")
    s.copy(out=ivh, in_=iv)

    tte = nc.gpsimd if GPS else v
    tt = st                                  # reuse [P, 2f] (st dead)
    ttj = tt.rearrange("p (j x) -> p j x", j=2)
    tte.tensor_sub(ttj,
                   s3.rearrange("p (j x) -> p j x", j=3)[:, 0:2, :],
                   _bc(s2v, 2))
    tte.tensor_mul(ttj, ttj, _bc(ivh, 2))

    # ---- Gram: cov6 = [c00|c11|c22|c01|c02|c12] ---------------------------
    # Off-diagonal chain first (independent of the ScalarE square of R6, so
    # the DVE overlaps it); diagonal chain after.
    sq6 = W(6 * f, tag="sq6")                # C entries squared (ScalarE)
    s.activation(sq6[:, 0:3 * f], r6[:, 0:3 * f], AF.Square)
    s.activation(sq6[:, 3 * f:6 * f], r6[:, 3 * f:6 * f], AF.Square)

    # v4 = [v00|v10|v01|v11] = t_j * (C0j, C1j)
    v4 = W(4 * f, tag="v4")
    r6j = r6.rearrange("p (j i x) -> p j i x", j=2, i=3)
    v.tensor_mul(v4.rearrange("p (j i x) -> p j i x", j=2, i=2),
                 r6j[:, :, 0:2, :],
                 ttj.unsqueeze(2).broadcast_to((P, 2, 2, f)))

    # pa = [v00*C10 | v00*C20 | v01*C11 | v01*C21]
    pa = sq4                                 # reuse [P, 4f] (sq4 dead)
    v4j = v4.rearrange("p (j i x) -> p j i x", j=2, i=2)
    v.tensor_mul(pa.rearrange("p (j i x) -> p j i x", j=2, i=2),
                 v4j[:, :, 0:1, :].broadcast_to((P, 2, 2, f)),
                 r6j[:, :, 1:3, :])
    # pb = [v10*C20 | v11*C21]
    pb = pq                                  # reuse [P, 2f] (pq dead)
    v.tensor_mul(pb.rearrange("p (j x) -> p j x", j=2),
                 v4j[:, :, 1:2, :].squeeze(2),
                 r6j[:, :, 2:3, :].squeeze(2))

    out6 = io.tile([P, 6 * f], FP16, tag="out6", name=f"out6_{t0}")
    out6e = out6.rearrange("p (e x) -> p e x", e=6)
    # off-diagonals; stream their DMA out as soon as they are done
    v.tensor_add(out6[:, 3 * f:5 * f], pa[:, 0:2 * f], pa[:, 2 * f:4 * f])
    v.tensor_add(out6[:, 5 * f:6 * f], pb[:, 0:f], pb[:, f:2 * f])
    nc.sync.dma_start(out=ov[:, 3:6, :], in_=out6e[:, 3:6, :])

    # diagonal: wsq = sq6 * t_j (in place, split by column j), then
    # wsq[0:3]+wsq[3:6] + s2
    sq6e = sq6.rearrange("p (e x) -> p e x", e=6)
    v.tensor_mul(sq6e[:, 0:3, :], sq6e[:, 0:3, :], _bc(tt[:, 0:f], 3))
    v.tensor_mul(sq6e[:, 3:6, :], sq6e[:, 3:6, :], _bc(tt[:, f:2 * f], 3))
    v.tensor_add(sq6[:, 0:3 * f], sq6[:, 0:3 * f], sq6[:, 3 * f:6 * f])
    (nc.gpsimd if GPS else v).tensor_add(
        out6e[:, 0:3, :],
        sq6.rearrange("p (e x) -> p e x", e=6)[:, 0:3, :],
        _bc(s2v, 3))
    nc.sync.dma_start(out=ov[:, 0:3, :], in_=out6e[:, 0:3, :])


def _prep_inputs(quaternion, log_scale):
    n = quaternion.shape[0]
    pad = N_CORES * NPC - n
    if pad:
        qpad = np.tile(np.array([1, 0, 0, 0], np.float32), (pad, 1))
        lpad = np.zeros((pad, 3), np.float32)
        quaternion = np.concatenate([quaternion, qpad], axis=0)
        log_scale = np.concatenate([log_scale, lpad], axis=0)
    in_maps = []
    for i in range(N_CORES):
        sl = slice(i * NPC, (i + 1) * NPC)
        # point index n = p*R + it*F + x  ->  tile-major [NT, P, c, F]
        qc = quaternion[sl].astype(np.float16).reshape(P, NT, F, 4)
        lc = log_scale[sl].astype(np.float16).reshape(P, NT, F, 3)
        in_maps.append({
            "q": np.ascontiguousarray(qc.transpose(1, 0, 3, 2)),
            "ls": np.ascontiguousarray(lc.transpose(1, 0, 3, 2)),
        })
    return in_maps


def kernel_with_stats(quaternion, log_scale, trace=False):
    quaternion = np.asarray(quaternion, dtype=np.float32)
    log_scale = np.asarray(log_scale, dtype=np.float32)
    n = quaternion.shape[0]
    nc = _build()
    in_maps = _prep_inputs(quaternion, log_scale)
    res = run_bass_kernel_spmd(nc, in_maps, core_ids=list(range(N_CORES)), trace=trace)
    # per-core cov6 is [NT, P, 6, F]; back to planes [6, n]
    planes = np.concatenate(
        [r["cov6"].transpose(2, 1, 0, 3).reshape(6, NPC) for r in res.results],
        axis=1)[:, :n]
    planes = planes.astype(np.float32)
    out = np.empty((n, 3, 3), np.float32)
    out[:, 0, 0] = planes[0]
    out[:, 1, 1] = planes[1]
    out[:, 2, 2] = planes[2]
    out[:, 0, 1] = out[:, 1, 0] = planes[3]
    out[:, 0, 2] = out[:, 2, 0] = planes[4]
    out[:, 1, 2] = out[:, 2, 1] = planes[5]
    return out, res


def kernel(quaternion, log_scale):
    out, _ = kernel_with_stats(quaternion, log_scale, trace=False)
    return out


# revision 24
# speedup vs baseline: 1.2751x; 1.0037x over previous
"""Trainium2 Bass kernel: per-point 3x3 Gaussian covariance from quaternion + log_scale.

cov = R diag(exp(log_scale)) R^T with R from the normalized quaternion.

Identity used (avoids normalizing q and the third rotation column):
  nu  = |q|^2 / 2
  C   = nu * R   (entries are plain quadratics of raw q: C00 = ha+hb-hc-hd,
                  C10 = bc+ad, C20 = bd-ac, C01 = bc-ad, C11 = ha-hb+hc-hd,
                  C21 = cd+ab, with hx = x^2/2)
  cov = s2*I + t0'*C0 C0^T + t1'*C1 C1^T,  tj' = (sj - s2)/nu^2
(uses R R^T = I to eliminate column 2.)

Layout: host uploads planar fp16 q[4, NPC], ls[3, NPC]; output is the 6
unique covariance entries planar fp16 cov6[6, NPC]; host symmetrizes and
upcasts.  On-chip everything is fp16 contiguous/block APs so every DVE
tensor_tensor hits the 2x_1p mode; squares/exp/copy run on ScalarE (all in
the one `exp_and_others` table set -> single ACT_TABLE_LOAD).  1/nu^2 via
the custom DVE reciprocal_approx_fast (no Ln, no table switch).
"""

import os
import numpy as np

import concourse.bass as bass
import concourse.bacc as bacc
import concourse.mybir as mybir
from concourse.tile import TileContext
from concourse.bass_utils import run_bass_kernel_spmd

AF = mybir.ActivationFunctionType
FP32 = mybir.dt.float32
FP16 = mybir.dt.float16

N_CORES = 8
N_FULL = 4_000_000
P = 128

F = int(os.environ.get("KERNEL_F", "978"))      # points per partition per tile
GPS = int(os.environ.get("KERNEL_GPSIMD", "0"))  # offload small ops to GPSIMD
                                                 # (measured: SBUF-port contention
                                                 # slows DVE more than it saves)
NT = -(-3907 // F)                               # tiles so that P*R*8 >= N
R = F * NT                                       # rows per partition per core
NPC = P * R                                      # points per core (padded)

SQRT_HALF = 0.7071067811865476

_built = {}


def _apv(t, off, pairs):
    """Raw AP view of tile t: keep its partition dim, replace free dims.

    pairs = [[stride, count], ...] in elements, offset in elements from the
    tile's base.
    """
    ap = [list(p) for p in t.ap]
    return bass.AP(tensor=t.tensor, offset=t.offset + off, ap=[ap[0]] + pairs)


def _bc(ap2d, n):
    """[P, f] -> [P, n, f] broadcast (stride-0 middle dim)."""
    p, f = ap2d.shape
    return ap2d.unsqueeze(1).broadcast_to((p, n, f))


def _build():
    key = F
    if key in _built:
        return _built[key]

    nc = bacc.Bacc("TRN2", target_bir_lowering=False, debug=False, num_devices=N_CORES)
    # Tile-major layouts: per (tile, partition) the data is contiguous in
    # DRAM, so every DMA descriptor is one large chunk per partition.
    q = nc.dram_tensor("q", [NT, P, 4, F], FP16, kind="ExternalInput")
    ls = nc.dram_tensor("ls", [NT, P, 3, F], FP16, kind="ExternalInput")
    cov = nc.dram_tensor("cov6", [NT, P, 6, F], FP16, kind="ExternalOutput")

    qv = q.ap()
    lsv = ls.ap()
    ov = cov.ap()

    with TileContext(nc) as tc:
        with (
            tc.tile_pool(name="io", bufs=2) as io,
            tc.tile_pool(name="wk", bufs=2) as wk,
        ):
            for it in range(NT):
                _tile_body(nc, io, wk, qv[it], lsv[it], ov[it], it * F, F)

    nc.compile()
    _built[key] = nc
    return nc


def _tile_body(nc, io, wk, qv, lsv, ov, t0, f):
    v = nc.vector
    s = nc.scalar

    def W(shape_f, dt=FP16, tag=None):
        return wk.tile([P, shape_f], dt, tag=tag, name=f"{tag}_{t0}")

    # ---- DMA in (q4 first; ls3 is only needed by the Exp much later) -----
    q4 = io.tile([P, 4 * f], FP16, tag="q4", name=f"q4_{t0}")
    ls3 = io.tile([P, 3 * f], FP16, tag="ls3", name=f"ls3_{t0}")
    nc.sync.dma_start(out=q4.rearrange("p (c x) -> p c x", c=4), in_=qv)

    # ---- half-squares + nu + diagonal R entries --------------------------
    sq4 = W(4 * f, tag="sq4_pa")            # [ha|hb|hc|hd]
    s.activation(sq4, q4, AF.Square, scale=SQRT_HALF)

    nc.sync.dma_start(out=ls3.rearrange("p (c x) -> p c x", c=3), in_=lsv)

    pq = W(2 * f, tag="pq_pb")              # [ha-hc | hb-hd]
    st = W(2 * f, tag="st_tt")              # [ha+hc | hb+hd]
    v.tensor_sub(pq, sq4[:, 0:2 * f], sq4[:, 2 * f:4 * f])
    v.tensor_add(st, sq4[:, 0:2 * f], sq4[:, 2 * f:4 * f])

    r6 = W(6 * f, tag="r6")                 # [C00|C10|C20|C01|C11|C21]
    v.tensor_add(r6[:, 0:f], pq[:, 0:f], pq[:, f:2 * f])          # C00
    v.tensor_sub(r6[:, 4 * f:5 * f], pq[:, 0:f], pq[:, f:2 * f])  # C11
    nu = W(f, FP16, tag="nu")
    v.tensor_add(nu, st[:, 0:f], st[:, f:2 * f])

    # ---- raw quaternion products (one [P,6f] tile: [ab|ac|ad|bc|bd|cd]) ---
    pq6 = W(6 * f, tag="pq6")
    pp = pq6[:, 0:3 * f]                    # [ab|ac|ad]
    qcd = pq6[:, 3 * f:6 * f]               # [bc|bd|cd]
    q4c = q4.rearrange("p (c x) -> p c x", c=4)
    v.tensor_mul(pp.rearrange("p (c x) -> p c x", c=3),
                 _bc(q4[:, 0:f], 3), q4c[:, 1:4, :])
    v.tensor_mul(qcd.rearrange("p (c x) -> p c x", c=3)[:, 0:2, :],
                 _bc(q4[:, f:2 * f], 2), q4c[:, 2:4, :])
    v.tensor_mul(qcd[:, 2 * f:3 * f], q4[:, 2 * f:3 * f], q4[:, 3 * f:4 * f])

    # ---- off-diagonal R entries (paired block ops) ------------------------
    # [C10|C21] = [bc|cd] + [ad|ab]
    v.tensor_add(_apv(r6, f, [[4 * f, 2], [1, f]]),
                 _apv(qcd, 0, [[2 * f, 2], [1, f]]),
                 _apv(pp, 2 * f, [[-2 * f, 2], [1, f]]))
    # [C20|C01] = [bd|bc] - [ac|ad]
    v.tensor_sub(_apv(r6, 2 * f, [[f, 2], [1, f]]),
                 _apv(qcd, f, [[-f, 2], [1, f]]),
                 _apv(pp, f, [[f, 2], [1, f]]))

    # ---- scales: s3 = exp(ls), tt = (sj - s2) / nu^2 ----------------------
    s3 = W(3 * f, tag="s3")
    s.activation(s3, ls3, AF.Exp)
    s2v = s3[:, 2 * f:3 * f]

    nusq = W(f, FP32, tag="nusq")
    s.activation(nusq, nu, AF.Square)
    iv = W(f, FP32, tag="iv")
    v.reciprocal_approx_fast(iv, nusq)
    ivh = W(f, FP16, tag="ivh")
    s.copy(out=ivh, in_=iv)

    tte = nc.gpsimd if GPS else v
    tt = st                                  # reuse [P, 2f] (st dead)
    ttj = tt.rearrange("p (j x) -> p j x", j=2)
    tte.tensor_sub(ttj,
                   s3.rearrange("p (j x) -> p j x", j=3)[:, 0:2, :],
                   _bc(s2v, 2))
    tte.tensor_mul(ttj, ttj, _bc(ivh, 2))

    # ---- Gram: cov6 = [c00|c11|c22|c01|c02|c12] ---------------------------
    # Off-diagonal chain first (independent of the ScalarE square of R6, so
    # the DVE overlaps it); diagonal chain after.
    sq6 = W(6 * f, tag="sq6")                # C entries squared (ScalarE)
    s.activation(sq6[:, 0:3 * f], r6[:, 0:3 * f], AF.Square)
    s.activation(sq6[:, 3 * f:6 * f], r6[:, 3 * f:6 * f], AF.Square)

    # v4 = [v00|v10|v01|v11] = t_j * (C0j, C1j)
    v4 = W(4 * f, tag="v4")
    r6j = r6.rearrange("p (j i x) -> p j i x", j=2, i=3)
    v.tensor_mul(v4.rearrange("p (j i x) -> p j i x", j=2, i=2),
                 r6j[:, :, 0:2, :],
                 ttj.unsqueeze(2).broadcast_to((P, 2, 2, f)))

    # pab = [v00*C10 | v00*C20 | v10*C20 || v01*C11 | v01*C21 | v11*C21]
    # (reuses pq6, dead after the cross ops; j-major so one 3f add below
    # yields [c01|c02|c12])
    pab = pq6
    v4j = v4.rearrange("p (j i x) -> p j i x", j=2, i=2)
    v.tensor_mul(pab.rearrange("p (j k x) -> p j k x", j=2, k=3)[:, :, 0:2, :],
                 v4j[:, :, 0:1, :].broadcast_to((P, 2, 2, f)),
                 r6j[:, :, 1:3, :])
    v.tensor_mul(_apv(pab, 2 * f, [[3 * f, 2], [1, f]]),
                 v4j[:, :, 1:2, :].squeeze(2),
                 r6j[:, :, 2:3, :].squeeze(2))

    out6 = io.tile([P, 6 * f], FP16, tag="out6", name=f"out6_{t0}")
    out6e = out6.rearrange("p (e x) -> p e x", e=6)
    # off-diagonals; stream their DMA out as soon as they are done
    v.tensor_add(out6[:, 3 * f:6 * f], pab[:, 0:3 * f], pab[:, 3 * f:6 * f])
    nc.sync.dma_start(out=ov[:, 3:6, :], in_=out6e[:, 3:6, :])

    # diagonal: wsq = sq6 * t_j (in place, split by column j), then
    # wsq[0:3]+wsq[3:6] + s2
    sq6e = sq6.rearrange("p (e x) -> p e x", e=6)
    v.tensor_mul(sq6e[:, 0:3, :], sq6e[:, 0:3, :], _bc(tt[:, 0:f], 3))
    v.tensor_mul(sq6e[:, 3:6, :], sq6e[:, 3:6, :], _bc(tt[:, f:2 * f], 3))
    v.tensor_add(sq6[:, 0:3 * f], sq6[:, 0:3 * f], sq6[:, 3 * f:6 * f])
    (nc.gpsimd if GPS else v).tensor_add(
        out6e[:, 0:3, :],
        sq6.rearrange("p (e x) -> p e x", e=6)[:, 0:3, :],
        _bc(s2v, 3))
    nc.sync.dma_start(out=ov[:, 0:3, :], in_=out6e[:, 0:3, :])


def _prep_inputs(quaternion, log_scale):
    n = quaternion.shape[0]
    pad = N_CORES * NPC - n
    if pad:
        qpad = np.tile(np.array([1, 0, 0, 0], np.float32), (pad, 1))
        lpad = np.zeros((pad, 3), np.float32)
        quaternion = np.concatenate([quaternion, qpad], axis=0)
        log_scale = np.concatenate([log_scale, lpad], axis=0)
    in_maps = []
    for i in range(N_CORES):
        sl = slice(i * NPC, (i + 1) * NPC)
        # point index n = p*R + it*F + x  ->  tile-major [NT, P, c, F]
        qc = quaternion[sl].astype(np.float16).reshape(P, NT, F, 4)
        lc = log_scale[sl].astype(np.float16).reshape(P, NT, F, 3)
        in_maps.append({
            "q": np.ascontiguousarray(qc.transpose(1, 0, 3, 2)),
            "ls": np.ascontiguousarray(lc.transpose(1, 0, 3, 2)),
        })
    return in_maps


def kernel_with_stats(quaternion, log_scale, trace=False):
    quaternion = np.asarray(quaternion, dtype=np.float32)
    log_scale = np.asarray(log_scale, dtype=np.float32)
    n = quaternion.shape[0]
    nc = _build()
    in_maps = _prep_inputs(quaternion, log_scale)
    res = run_bass_kernel_spmd(nc, in_maps, core_ids=list(range(N_CORES)), trace=trace)
    # per-core cov6 is [NT, P, 6, F]; back to planes [6, n]
    planes = np.concatenate(
        [r["cov6"].transpose(2, 1, 0, 3).reshape(6, NPC) for r in res.results],
        axis=1)[:, :n]
    planes = planes.astype(np.float32)
    out = np.empty((n, 3, 3), np.float32)
    out[:, 0, 0] = planes[0]
    out[:, 1, 1] = planes[1]
    out[:, 2, 2] = planes[2]
    out[:, 0, 1] = out[:, 1, 0] = planes[3]
    out[:, 0, 2] = out[:, 2, 0] = planes[4]
    out[:, 1, 2] = out[:, 2, 1] = planes[5]
    return out, res


def kernel(quaternion, log_scale):
    out, _ = kernel_with_stats(quaternion, log_scale, trace=False)
    return out


# revision 25
# speedup vs baseline: 1.2874x; 1.0096x over previous
"""Trainium2 Bass kernel: per-point 3x3 Gaussian covariance from quaternion + log_scale.

cov = R diag(exp(log_scale)) R^T with R from the normalized quaternion.

Identity used (avoids normalizing q and the third rotation column):
  nu  = |q|^2 / 2
  C   = nu * R   (entries are plain quadratics of raw q: C00 = ha+hb-hc-hd,
                  C10 = bc+ad, C20 = bd-ac, C01 = bc-ad, C11 = ha-hb+hc-hd,
                  C21 = cd+ab, with hx = x^2/2)
  cov = s2*I + t0'*C0 C0^T + t1'*C1 C1^T,  tj' = (sj - s2)/nu^2
(uses R R^T = I to eliminate column 2.)

Layout: host uploads planar fp16 q[4, NPC], ls[3, NPC]; output is the 6
unique covariance entries planar fp16 cov6[6, NPC]; host symmetrizes and
upcasts.  On-chip everything is fp16 contiguous/block APs so every DVE
tensor_tensor hits the 2x_1p mode; squares/exp/copy run on ScalarE (all in
the one `exp_and_others` table set -> single ACT_TABLE_LOAD).  1/nu^2 via
the custom DVE reciprocal_approx_fast (no Ln, no table switch).
"""

import os
import numpy as np

import concourse.bass as bass
import concourse.bacc as bacc
import concourse.mybir as mybir
from concourse.tile import TileContext
from concourse.bass_utils import run_bass_kernel_spmd

AF = mybir.ActivationFunctionType
FP32 = mybir.dt.float32
FP16 = mybir.dt.float16

N_CORES = 8
N_FULL = 4_000_000
P = 128

F = int(os.environ.get("KERNEL_F", "978"))      # points per partition per tile
GPS = int(os.environ.get("KERNEL_GPSIMD", "0"))  # offload small ops to GPSIMD
                                                 # (measured: SBUF-port contention
                                                 # slows DVE more than it saves)
NT = -(-3907 // F)                               # tiles so that P*R*8 >= N
R = F * NT                                       # rows per partition per core
NPC = P * R                                      # points per core (padded)

SQRT_HALF = 0.7071067811865476

_built = {}


def _apv(t, off, pairs):
    """Raw AP view of tile t: keep its partition dim, replace free dims.

    pairs = [[stride, count], ...] in elements, offset in elements from the
    tile's base.
    """
    ap = [list(p) for p in t.ap]
    return bass.AP(tensor=t.tensor, offset=t.offset + off, ap=[ap[0]] + pairs)


def _bc(ap2d, n):
    """[P, f] -> [P, n, f] broadcast (stride-0 middle dim)."""
    p, f = ap2d.shape
    return ap2d.unsqueeze(1).broadcast_to((p, n, f))


def _build():
    key = F
    if key in _built:
        return _built[key]

    nc = bacc.Bacc("TRN2", target_bir_lowering=False, debug=False, num_devices=N_CORES)
    # Tile-major layouts: per (tile, partition) the data is contiguous in
    # DRAM, so every DMA descriptor is one large chunk per partition.
    q = nc.dram_tensor("q", [NT, P, 4, F], FP16, kind="ExternalInput")
    ls = nc.dram_tensor("ls", [NT, P, 3, F], FP16, kind="ExternalInput")
    cov = nc.dram_tensor("cov6", [NT, P, 6, F], FP16, kind="ExternalOutput")

    qv = q.ap()
    lsv = ls.ap()
    ov = cov.ap()

    with TileContext(nc) as tc:
        with (
            tc.tile_pool(name="io", bufs=2) as io,
            tc.tile_pool(name="wk", bufs=2) as wk,
        ):
            for it in range(NT):
                _tile_body(nc, io, wk, qv[it], lsv[it], ov[it], it * F, F)

    nc.compile()
    _built[key] = nc
    return nc


def _tile_body(nc, io, wk, qv, lsv, ov, t0, f):
    v = nc.vector
    s = nc.scalar

    def W(shape_f, dt=FP16, tag=None):
        return wk.tile([P, shape_f], dt, tag=tag, name=f"{tag}_{t0}")

    # ---- DMA in (q4 first; ls3 is only needed by the Exp much later) -----
    q4 = io.tile([P, 4 * f], FP16, tag="q4", name=f"q4_{t0}")
    ls3 = io.tile([P, 3 * f], FP16, tag="ls3", name=f"ls3_{t0}")
    nc.sync.dma_start(out=q4.rearrange("p (c x) -> p c x", c=4), in_=qv)

    # ---- half-squares + nu + diagonal R entries --------------------------
    sq4 = W(4 * f, tag="sq4_pa")            # [ha|hb|hc|hd]
    s.activation(sq4, q4, AF.Square, scale=SQRT_HALF)

    nc.sync.dma_start(out=ls3.rearrange("p (c x) -> p c x", c=3), in_=lsv)

    pq = W(2 * f, tag="pq_pb")              # [ha-hc | hb-hd]
    st = W(2 * f, tag="st_tt")              # [ha+hc | hb+hd]
    v.tensor_sub(pq, sq4[:, 0:2 * f], sq4[:, 2 * f:4 * f])
    v.tensor_add(st, sq4[:, 0:2 * f], sq4[:, 2 * f:4 * f])

    r6 = W(6 * f, tag="r6")                 # [C00|C10|C20|C01|C11|C21]
    v.tensor_add(r6[:, 0:f], pq[:, 0:f], pq[:, f:2 * f])          # C00
    v.tensor_sub(r6[:, 4 * f:5 * f], pq[:, 0:f], pq[:, f:2 * f])  # C11
    nu = W(f, FP16, tag="nu")
    v.tensor_add(nu, st[:, 0:f], st[:, f:2 * f])

    # ---- raw quaternion products (one [P,6f] tile: [ab|ac|ad|bc|bd|cd]) ---
    pq6 = W(6 * f, tag="pq6")
    pp = pq6[:, 0:3 * f]                    # [ab|ac|ad]
    qcd = pq6[:, 3 * f:6 * f]               # [bc|bd|cd]
    q4c = q4.rearrange("p (c x) -> p c x", c=4)
    v.tensor_mul(pp.rearrange("p (c x) -> p c x", c=3),
                 _bc(q4[:, 0:f], 3), q4c[:, 1:4, :])
    v.tensor_mul(qcd.rearrange("p (c x) -> p c x", c=3)[:, 0:2, :],
                 _bc(q4[:, f:2 * f], 2), q4c[:, 2:4, :])
    v.tensor_mul(qcd[:, 2 * f:3 * f], q4[:, 2 * f:3 * f], q4[:, 3 * f:4 * f])

    # ---- scales: s3 = exp(ls), iv = 1 / nu^2 ------------------------------
    # (emitted between the product ops so the ScalarE round-trips
    # nusq -> recip -> ivh hide under DVE work)
    s3 = W(3 * f, tag="s3")
    s.activation(s3, ls3, AF.Exp)
    s2v = s3[:, 2 * f:3 * f]
    nusq = W(f, FP32, tag="nusq")
    s.activation(nusq, nu, AF.Square)
    iv = W(f, FP32, tag="iv")
    v.reciprocal_approx_fast(iv, nusq)
    ivh = W(f, FP16, tag="ivh")
    s.copy(out=ivh, in_=iv)

    # ---- off-diagonal R entries (paired block ops) ------------------------
    # [C10|C21] = [bc|cd] + [ad|ab]
    v.tensor_add(_apv(r6, f, [[4 * f, 2], [1, f]]),
                 _apv(qcd, 0, [[2 * f, 2], [1, f]]),
                 _apv(pp, 2 * f, [[-2 * f, 2], [1, f]]))
    # [C20|C01] = [bd|bc] - [ac|ad]
    v.tensor_sub(_apv(r6, 2 * f, [[f, 2], [1, f]]),
                 _apv(qcd, f, [[-f, 2], [1, f]]),
                 _apv(pp, f, [[f, 2], [1, f]]))

    # ---- tt = (sj - s2) * iv ----------------------------------------------

    tte = nc.gpsimd if GPS else v
    tt = st                                  # reuse [P, 2f] (st dead)
    ttj = tt.rearrange("p (j x) -> p j x", j=2)
    tte.tensor_sub(ttj,
                   s3.rearrange("p (j x) -> p j x", j=3)[:, 0:2, :],
                   _bc(s2v, 2))
    tte.tensor_mul(ttj, ttj, _bc(ivh, 2))

    # ---- Gram: cov6 = [c00|c11|c22|c01|c02|c12] ---------------------------
    # Off-diagonal chain first (independent of the ScalarE square of R6, so
    # the DVE overlaps it); diagonal chain after.
    sq6 = W(6 * f, tag="sq6")                # C entries squared (ScalarE)
    s.activation(sq6[:, 0:3 * f], r6[:, 0:3 * f], AF.Square)
    s.activation(sq6[:, 3 * f:6 * f], r6[:, 3 * f:6 * f], AF.Square)

    # v4 = [v00|v10|v01|v11] = t_j * (C0j, C1j)
    v4 = W(4 * f, tag="v4")
    r6j = r6.rearrange("p (j i x) -> p j i x", j=2, i=3)
    v.tensor_mul(v4.rearrange("p (j i x) -> p j i x", j=2, i=2),
                 r6j[:, :, 0:2, :],
                 ttj.unsqueeze(2).broadcast_to((P, 2, 2, f)))

    # pab = [v00*C10 | v00*C20 | v10*C20 || v01*C11 | v01*C21 | v11*C21]
    # (reuses pq6, dead after the cross ops; j-major so one 3f add below
    # yields [c01|c02|c12])
    pab = pq6
    v4j = v4.rearrange("p (j i x) -> p j i x", j=2, i=2)
    v.tensor_mul(pab.rearrange("p (j k x) -> p j k x", j=2, k=3)[:, :, 0:2, :],
                 v4j[:, :, 0:1, :].broadcast_to((P, 2, 2, f)),
                 r6j[:, :, 1:3, :])
    v.tensor_mul(_apv(pab, 2 * f, [[3 * f, 2], [1, f]]),
                 v4j[:, :, 1:2, :].squeeze(2),
                 r6j[:, :, 2:3, :].squeeze(2))

    out6 = io.tile([P, 6 * f], FP16, tag="out6", name=f"out6_{t0}")
    out6e = out6.rearrange("p (e x) -> p e x", e=6)
    # off-diagonals; stream their DMA out as soon as they are done
    v.tensor_add(out6[:, 3 * f:6 * f], pab[:, 0:3 * f], pab[:, 3 * f:6 * f])
    nc.sync.dma_start(out=ov[:, 3:6, :], in_=out6e[:, 3:6, :])

    # diagonal: wsq = sq6 * t_j (in place, split by column j), then
    # wsq[0:3]+wsq[3:6] + s2
    sq6e = sq6.rearrange("p (e x) -> p e x", e=6)
    v.tensor_mul(sq6e[:, 0:3, :], sq6e[:, 0:3, :], _bc(tt[:, 0:f], 3))
    v.tensor_mul(sq6e[:, 3:6, :], sq6e[:, 3:6, :], _bc(tt[:, f:2 * f], 3))
    v.tensor_add(sq6[:, 0:3 * f], sq6[:, 0:3 * f], sq6[:, 3 * f:6 * f])
    (nc.gpsimd if GPS else v).tensor_add(
        out6e[:, 0:3, :],
        sq6.rearrange("p (e x) -> p e x", e=6)[:, 0:3, :],
        _bc(s2v, 3))
    nc.sync.dma_start(out=ov[:, 0:3, :], in_=out6e[:, 0:3, :])


def _prep_inputs(quaternion, log_scale):
    n = quaternion.shape[0]
    pad = N_CORES * NPC - n
    if pad:
        qpad = np.tile(np.array([1, 0, 0, 0], np.float32), (pad, 1))
        lpad = np.zeros((pad, 3), np.float32)
        quaternion = np.concatenate([quaternion, qpad], axis=0)
        log_scale = np.concatenate([log_scale, lpad], axis=0)
    in_maps = []
    for i in range(N_CORES):
        sl = slice(i * NPC, (i + 1) * NPC)
        # point index n = p*R + it*F + x  ->  tile-major [NT, P, c, F]
        qc = quaternion[sl].astype(np.float16).reshape(P, NT, F, 4)
        lc = log_scale[sl].astype(np.float16).reshape(P, NT, F, 3)
        in_maps.append({
            "q": np.ascontiguousarray(qc.transpose(1, 0, 3, 2)),
            "ls": np.ascontiguousarray(lc.transpose(1, 0, 3, 2)),
        })
    return in_maps


def kernel_with_stats(quaternion, log_scale, trace=False):
    quaternion = np.asarray(quaternion, dtype=np.float32)
    log_scale = np.asarray(log_scale, dtype=np.float32)
    n = quaternion.shape[0]
    nc = _build()
    in_maps = _prep_inputs(quaternion, log_scale)
    res = run_bass_kernel_spmd(nc, in_maps, core_ids=list(range(N_CORES)), trace=trace)
    # per-core cov6 is [NT, P, 6, F]; back to planes [6, n]
    planes = np.concatenate(
        [r["cov6"].transpose(2, 1, 0, 3).reshape(6, NPC) for r in res.results],
        axis=1)[:, :n]
    planes = planes.astype(np.float32)
    out = np.empty((n, 3, 3), np.float32)
    out[:, 0, 0] = planes[0]
    out[:, 1, 1] = planes[1]
    out[:, 2, 2] = planes[2]
    out[:, 0, 1] = out[:, 1, 0] = planes[3]
    out[:, 0, 2] = out[:, 2, 0] = planes[4]
    out[:, 1, 2] = out[:, 2, 1] = planes[5]
    return out, res


def kernel(quaternion, log_scale):
    out, _ = kernel_with_stats(quaternion, log_scale, trace=False)
    return out
